# revision 1
# baseline (speedup 1.0000x reference)
"""Trainium2 Bass kernel for 3-layer GraphSAGE (nn_MCHCGraphSage).

Strategy (8 NeuronCores, SPMD single program):
  - Destination-sharded edges: core k owns dst nodes [k*6250, (k+1)*6250).
  - Features live in HBM as 256B rows addressed by "padded slab address"
    addr(n) = n + 22*(n//6250)  (8 slabs of 6272 rows = 50176 rows), which
    makes the inter-layer AllGather output land directly in gather space.
  - Random x[src] rows are fetched with gpsimd dma_gather (int16 indices).
    int16 range forces a two-section split: section A gathers rows
    [0, 32768), section B gathers rows [17408, 50176) (base offset 17408).
  - Segmented mean via two PE matmul levels over dst-sorted, degree-padded
    (multiple of 4) edge slots:
      level 1: constant block-ones lhsT [128, 32] sums groups of 4 slots
               into PSUM rows (two chunks per 64-group block).
      level 2: host-built 0/1 one-hot [64 groups, 128 dst] (bf16) maps
               groups to dst columns of a 128-dst window, accumulated in
               PSUM; per-group scale (1/deg) applied between levels.
  - Dense part per window, node-major: y = meanT.T @ Wl + hselfT.T @ Ws_ext
    (bias folded as an extra ones-row of hselfT), ReLU on ACT, DMA the
    [128, 64] node-major block straight to the own slab; PE-transpose to
    keep the feature-major self slab for the next layer. AllGather between
    layers redistributes slabs.
"""

import os
import sys

import numpy as np

for _p in ("/opt/trn_rl_repo", "/root/.axon_site/_ro/trn_rl_repo"):
    if os.path.isdir(_p) and _p not in sys.path:
        sys.path.append(_p)

import ml_dtypes  # noqa: E402

N = 50000
D = 64
NCORES = 8
SLAB = 6250
PSLAB = 6272
WIN = 128
NW = PSLAB // WIN  # 49
TOTROW = NCORES * PSLAB  # 50176
BBASE = TOTROW - 32768  # 17408, B-section base row
APAD_ROW = SLAB  # row 6250 (core 0 slab padding) is always zero
BPAD_ROW = 3 * PSLAB + SLAB  # row 25066 (core 3 slab padding) always zero
BW = 4  # windows per gather batch

_NC_CACHE = {}
LAST_RESULTS = None  # test harness introspection (exec_time_ns, profile)


def _addr(n):
    return n + 22 * (n // SLAB)


def _pack(x, edge_index, scale, use_bf16):
    """Host-side packing. Returns per-core dicts + structure constants."""
    src = np.asarray(edge_index[0], dtype=np.int64)
    dst = np.asarray(edge_index[1], dtype=np.int64)
    addr_e = _addr(src)

    # pass 1: global section sizes
    nch_a = 0
    nch_b = 0
    per_core = []
    for k in range(NCORES):
        sel = (dst >= k * SLAB) & (dst < (k + 1) * SLAB)
        s_k = src[sel]
        d_k = dst[sel] - k * SLAB
        a_k = addr_e[sel]
        isA = a_k <= 32767
        degA = np.bincount(d_k[isA], minlength=PSLAB)
        degB = np.bincount(d_k[~isA], minlength=PSLAB)
        padA = ((degA + 3) // 4) * 4
        padB = ((degB + 3) // 4) * 4
        wA = padA.reshape(NW, WIN).sum(1).max()
        wB = padB.reshape(NW, WIN).sum(1).max()
        nch_a = max(nch_a, (int(wA) + 127) // 128)
        nch_b = max(nch_b, (int(wB) + 127) // 128)
        per_core.append((d_k, a_k, isA, padA, padB))

    S_A = nch_a * 128
    S_B = nch_b * 128
    NCH = nch_a + nch_b
    GW = 32 * NCH  # groups per window
    NBLK = (NCH + 3) // 4  # 128-group blocks per window
    fdt = ml_dtypes.bfloat16 if use_bf16 else np.float32
    ROW = 128 if use_bf16 else 64

    # xext: node features in padded-slab address space, same for all cores
    xext = np.zeros((TOTROW, ROW), dtype=fdt)
    rows = _addr(np.arange(N))
    xext[rows, :D] = x.astype(fdt)

    cores = []
    for k in range(NCORES):
        d_k, a_k, isA, padA, padB = per_core[k]
        pA2 = padA.reshape(NW, WIN)
        pB2 = padB.reshape(NW, WIN)
        offA = (np.cumsum(pA2, 1) - pA2).reshape(-1)  # per local dst
        offB = (np.cumsum(pB2, 1) - pB2).reshape(-1)

        def build_stream(mask, off, S, base, padval):
            e_d = d_k[mask]
            e_a = a_k[mask]
            order = np.argsort(e_d, kind="stable")
            d_s = e_d[order]
            a_s = e_a[order]
            deg = np.bincount(e_d, minlength=PSLAB)
            start = np.concatenate([[0], np.cumsum(deg)])[:-1]
            rank = np.arange(len(d_s)) - start[d_s]
            pos = (d_s // WIN) * S + off[d_s] + rank
            stream = np.full(NW * S, padval, dtype=np.int64)
            stream[pos] = a_s - base
            return stream

        streamA = build_stream(isA, offA, S_A, 0, APAD_ROW)
        streamB = build_stream(~isA, offB, S_B, BBASE, BPAD_ROW - BBASE)
        assert streamA.max() <= 32767 and streamB.max() <= 32767
        assert streamA.min() >= 0 and streamB.min() >= 0

        # group -> local dst map (per window: A groups then B groups)
        grp_dst = np.full(NW * GW, -1, dtype=np.int64)
        for pad, off, sec0 in ((padA, offA, 0), (padB, offB, S_A)):
            reps = pad // 4
            tot = int(reps.sum())
            if tot == 0:
                continue
            gstart = (np.arange(PSLAB) // WIN) * GW + (sec0 + off) // 4
            base_rep = np.repeat(gstart, reps)
            cum = np.cumsum(reps) - reps
            within = np.arange(tot) - np.repeat(cum, reps)
            gpos = base_rep + within
            grp_dst[gpos] = np.repeat(np.arange(PSLAB), reps)

        valid = grp_dst >= 0
        gw = np.arange(NW * GW) // GW
        gl = np.arange(NW * GW) % GW
        onehot = np.zeros((128, NW * NBLK * 128), dtype=ml_dtypes.bfloat16)
        ocols = (gw * NBLK + gl // 128) * 128 + (grp_dst % WIN)
        onehot[gl[valid] % 128, ocols[valid]] = 1.0
        sgrp = np.zeros((128, NW * NBLK), dtype=np.float32)
        sgrp[gl[valid] % 128, (gw * NBLK + gl // 128)[valid]] = scale[
            k * SLAB + grp_dst[valid]
        ]

        stream = np.concatenate([streamA, streamB]).astype(np.int16)
        idx16 = stream.reshape(-1, 16).T.copy()  # [16, T/16]
        idx = np.tile(idx16, (8, 1))  # replicate for 8 gpsimd cores

        xselfT = np.zeros((D + 1, PSLAB), dtype=fdt)
        xselfT[:D, :SLAB] = x[k * SLAB : (k + 1) * SLAB].T.astype(fdt)
        xselfT[D, :] = 1.0  # bias row

        cores.append(
            {"idx": idx, "onehot": onehot, "sgrp": sgrp, "xselfT": xselfT}
        )

    return nch_a, nch_b, NBLK, xext, cores


def _build_nc(nch_a, nch_b, nblk, use_bf16):
    import concourse.bacc as bacc
    import concourse.tile as tile
    import concourse.mybir as mybir

    dt = mybir.dt
    fdt = dt.bfloat16 if use_bf16 else dt.float32
    ROW = 128 if use_bf16 else 64
    NCH = nch_a + nch_b
    S_A = nch_a * 128
    S_B = nch_b * 128
    T_A = NW * S_A
    T_B = NW * S_B

    nqueues = int(os.environ.get("SAGE_QUEUES", "1"))
    nc = bacc.Bacc(None, num_devices=NCORES, num_swdge_queues=nqueues)

    xext_d = nc.dram_tensor("xext", [TOTROW, ROW], fdt, kind="ExternalInput")
    idx_d = nc.dram_tensor(
        "idx", [128, (T_A + T_B) // 16], dt.int16, kind="ExternalInput"
    )
    oh_d = nc.dram_tensor(
        "onehot", [128, NW * nblk * 128], dt.bfloat16, kind="ExternalInput"
    )
    sg_d = nc.dram_tensor("sgrp", [128, NW * nblk], dt.float32, kind="ExternalInput")
    xsT_d = nc.dram_tensor("xselfT", [D + 1, PSLAB], fdt, kind="ExternalInput")
    bones_d = nc.dram_tensor("bones", [128, 32], fdt, kind="ExternalInput")
    r0_last = (NCH - 1) % 4
    M_LAST = 128 - 32 * r0_last if r0_last == 0 else 32
    bonesl_d = nc.dram_tensor("bonesl", [128, M_LAST], fdt, kind="ExternalInput")
    ident_d = nc.dram_tensor("ident", [WIN, WIN], fdt, kind="ExternalInput")
    w_d = {}
    for l, m in ((0, D), (1, D), (2, 1)):
        w_d[f"wl{l}"] = nc.dram_tensor(f"wl{l}", [D, m], fdt, kind="ExternalInput")
        w_d[f"ws{l}"] = nc.dram_tensor(
            f"ws{l}", [D + 1, m], fdt, kind="ExternalInput"
        )
    out_d = nc.dram_tensor("out", [PSLAB, 1], dt.float32, kind="ExternalOutput")

    hext_d = nc.dram_tensor("hext", [TOTROW, ROW], fdt, addr_space="Shared")
    slab_d = nc.dram_tensor("slab", [PSLAB, ROW], fdt)

    batches = []
    w0 = 0
    while w0 < NW:
        bw = min(BW, NW - w0)
        batches.append((w0, bw))
        w0 += bw
    n_layers = int(os.environ.get("SAGE_LAYERS", "3"))
    n_batch_lim = int(os.environ.get("SAGE_BATCHES", str(len(batches))))
    batches = batches[:n_batch_lim]
    no_cc = os.environ.get("SAGE_NOCC", "") == "1"

    with tile.TileContext(nc) as tc:
        with (
            tc.tile_pool(name="const", bufs=1) as cpool,
            tc.tile_pool(name="gpool", bufs=2) as gpool,
            tc.tile_pool(name="spool", bufs=3) as spool,
            tc.tile_pool(name="psA", bufs=2, space="PSUM") as psA,
            tc.tile_pool(name="psB", bufs=2, space="PSUM") as psB,
            tc.tile_pool(name="psC", bufs=2, space="PSUM") as psC,
        ):
            idx_sb = cpool.tile([128, (T_A + T_B) // 16], dt.int16, tag="idx")
            oh_sb = cpool.tile([128, NW * nblk * 128], dt.bfloat16, tag="oh")
            sg_sb = cpool.tile([128, NW * nblk], dt.float32, tag="sg")
            bones_sb = cpool.tile([128, 32], fdt, tag="bones")
            bonesl_sb = cpool.tile([128, M_LAST], fdt, tag="bonesl")
            zeros_sb = cpool.tile([128, 64], fdt, tag="zeros")
            ident_sb = cpool.tile([WIN, WIN], fdt, tag="ident")
            hs = [cpool.tile([D + 1, PSLAB], fdt, tag=f"hs{i}", name=f"hs{i}")
                  for i in range(3)]
            w_sb = {}
            for l, m in ((0, D), (1, D), (2, 1)):
                w_sb[f"wl{l}"] = cpool.tile([D, m], fdt, tag=f"wl{l}",
                                            name=f"wl{l}")
                w_sb[f"ws{l}"] = cpool.tile([D + 1, m], fdt, tag=f"ws{l}",
                                            name=f"ws{l}")
            zpad_sb = cpool.tile([PSLAB - SLAB, ROW], fdt, tag="zpad")

            nc.sync.dma_start(idx_sb[:], idx_d[:])
            nc.sync.dma_start(oh_sb[:], oh_d[:])
            nc.sync.dma_start(sg_sb[:], sg_d[:])
            nc.sync.dma_start(bones_sb[:], bones_d[:])
            nc.sync.dma_start(bonesl_sb[:], bonesl_d[:])
            nc.sync.dma_start(ident_sb[:], ident_d[:])
            nc.sync.dma_start(hs[0][:], xsT_d[:])
            for l in range(3):
                nc.sync.dma_start(w_sb[f"wl{l}"][:], w_d[f"wl{l}"][:])
                nc.sync.dma_start(w_sb[f"ws{l}"][:], w_d[f"ws{l}"][:])
            nc.vector.memset(zpad_sb[:], 0.0)
            nc.vector.memset(zeros_sb[:], 0.0)
            nc.vector.memset(hs[1][D : D + 1, :], 1.0)
            nc.vector.memset(hs[2][D : D + 1, :], 1.0)

            import contextlib
            reps = int(os.environ.get("SAGE_REPS", "1"))
            rep_cm = (tc.For_i(0, reps, 1, name="reploop")
                      if reps > 1 else contextlib.nullcontext())
            with rep_cm:
                for layer in range(n_layers):
                    src_t = xext_d if layer == 0 else hext_d
                    hself = hs[layer]
                    wl_t = w_sb[f"wl{layer}"]
                    ws_t = w_sb[f"ws{layer}"]
                    m_out = 1 if layer == 2 else D

                    for bi, (w0, bw) in enumerate(batches):
                        gA = gpool.tile([128, bw * nch_a, ROW], fdt, tag="gA")
                        gB = gpool.tile([128, bw * nch_b, ROW], fdt, tag="gB")
                        numA = bw * S_A
                        numB = bw * S_B
                        a0 = w0 * S_A // 16
                        b0c = (T_A + w0 * S_B) // 16
                        nc.gpsimd.dma_gather(
                            gA[:], xext_d[:] if layer == 0 else hext_d[:],
                            idx_sb[:, a0 : a0 + numA // 16],
                            numA, numA, ROW,
                            single_packet=False,
                        )
                        nc.gpsimd.dma_gather(
                            gB[:], src_t[BBASE:, :],
                            idx_sb[:, b0c : b0c + numB // 16],
                            numB, numB, ROW,
                            single_packet=False,
                        )

                        stage = int(os.environ.get("SAGE_STAGE", "9"))
                        for wi in range(bw):
                            if stage < 1:
                                break
                            w = w0 + wi
                            gsum_ps = psA.tile([128, nblk * D], dt.float32, tag="gsum")
                            # level 1: block-ones partial sums (groups of 4 slots)
                            for cc in range(NCH):
                                if cc < nch_a:
                                    rhs = gA[:, wi * nch_a + cc, 0:D]
                                else:
                                    rhs = gB[:, wi * nch_b + (cc - nch_a), 0:D]
                                blk = cc // 4
                                row = (cc % 4) * 32
                                cslice = slice(blk * D, (blk + 1) * D)
                                if cc == NCH - 1 and r0_last == 0:
                                    # covers rows [0,128): tail rows zero-filled
                                    nc.tensor.matmul(
                                        gsum_ps[0:128, cslice],
                                        bonesl_sb[:], rhs, start=True, stop=True,
                                    )
                                else:
                                    nc.tensor.matmul(
                                        gsum_ps[row : row + 32, cslice],
                                        bones_sb[:], rhs, start=True, stop=True,
                                        tile_position=(0, row),
                                    )
                                    if cc == NCH - 1 and row < 96:
                                        # zero-fill remaining rows of last block
                                        z0 = row + 32
                                        if z0 == 32:
                                            nc.tensor.matmul(
                                                gsum_ps[32:64, cslice],
                                                zeros_sb[:, 0:32], rhs,
                                                start=True, stop=True,
                                                tile_position=(0, 32),
                                            )
                                            z0 = 64
                                        if z0 == 64:
                                            nc.tensor.matmul(
                                                gsum_ps[64:128, cslice],
                                                zeros_sb[:], rhs,
                                                start=True, stop=True,
                                                tile_position=(0, 64),
                                            )
                                        elif z0 == 96:
                                            nc.tensor.matmul(
                                                gsum_ps[96:128, cslice],
                                                zeros_sb[:, 0:32], rhs,
                                                start=True, stop=True,
                                                tile_position=(0, 96),
                                            )
                            if stage < 2:
                                continue
                            # scale by 1/deg, cast to bf16 (split over ACT/DVE)
                            gsum_sb = spool.tile([128, nblk * D], dt.bfloat16,
                                                 tag="gsum_sb")
                            for blk in range(nblk):
                                sga = sg_sb[:, w * nblk + blk : w * nblk + blk + 1]
                                if blk % 2 == 0:
                                    nc.scalar.activation(
                                        gsum_sb[:, blk * D : (blk + 1) * D],
                                        gsum_ps[:, blk * D : (blk + 1) * D],
                                        mybir.ActivationFunctionType.Copy,
                                        scale=sga,
                                    )
                                else:
                                    nc.vector.tensor_scalar_mul(
                                        gsum_sb[:, blk * D : (blk + 1) * D],
                                        gsum_ps[:, blk * D : (blk + 1) * D],
                                        sga,
                                    )
                            if stage < 3:
                                continue
                            # level 2: one-hot accumulate -> meanT [D, 128] scaled
                            win_ps = psB.tile([D, WIN], dt.float32, tag="winps")
                            for blk in range(nblk):
                                oc = (w * nblk + blk) * 128
                                nc.tensor.matmul(
                                    win_ps[:],
                                    gsum_sb[:, blk * D : (blk + 1) * D],
                                    oh_sb[:, oc : oc + 128],
                                    start=(blk == 0), stop=(blk == nblk - 1),
                                )
                            if stage < 4:
                                continue
                            mean_sb = spool.tile([D, WIN], fdt, tag="mean")
                            nc.vector.tensor_copy(mean_sb[:], win_ps[:])
                            # dense, node-major: y = meanT.T@Wl + hselfT.T@Ws_ext
                            y_ps = psC.tile([WIN, m_out], dt.float32, tag="ypsum")
                            nc.tensor.matmul(y_ps[:], mean_sb[:], wl_t[:],
                                             start=True, stop=False)
                            nc.tensor.matmul(y_ps[:],
                                             hself[:, w * WIN : (w + 1) * WIN],
                                             ws_t[:], start=False, stop=True)
                            if layer < 2:
                                hn_sb = spool.tile([WIN, D], fdt, tag="hn")
                                nc.scalar.activation(
                                    hn_sb[:], y_ps[:],
                                    mybir.ActivationFunctionType.Relu,
                                )
                                nc.sync.dma_start(
                                    slab_d[w * WIN : (w + 1) * WIN, 0:D], hn_sb[:]
                                )
                                t_ps = psB.tile([D, WIN], fdt, tag="tps",
                                                name="t_ps")
                                nc.tensor.transpose(t_ps[:], hn_sb[:], ident_sb[:])
                                nc.vector.tensor_copy(
                                    hs[layer + 1][0:D, w * WIN : (w + 1) * WIN],
                                    t_ps[:],
                                )
                            else:
                                y_sb = spool.tile([WIN, 1], dt.float32, tag="ysb")
                                nc.scalar.activation(
                                    y_sb[:], y_ps[:],
                                    mybir.ActivationFunctionType.Relu,
                                )
                                nc.sync.dma_start(
                                    out_d[w * WIN : (w + 1) * WIN, :], y_sb[:]
                                )

                    if layer < 2 and layer < n_layers - 1 and not no_cc:
                        nc.sync.dma_start(slab_d[SLAB:PSLAB, :], zpad_sb[:])
                        nc.gpsimd.collective_compute(
                            "AllGather",
                            mybir.AluOpType.bypass,
                            replica_groups=[list(range(NCORES))],
                            ins=[slab_d[:]],
                            outs=[hext_d[:]],
                        )

    nc.compile()
    return nc


def kernel(**inputs):
    x = np.asarray(inputs["x"], dtype=np.float32)
    edge_index = np.asarray(inputs["edge_index"])
    use_bf16 = os.environ.get("SAGE_F32", "") != "1"

    deg = np.bincount(np.asarray(edge_index[1], dtype=np.int64), minlength=N)
    scale = np.where(deg > 0, 1.0 / np.maximum(deg, 1), 0.0).astype(np.float32)

    nch_a, nch_b, nblk, xext, cores = _pack(x, edge_index, scale, use_bf16)

    key = (nch_a, nch_b, nblk, use_bf16)
    if key not in _NC_CACHE:
        _NC_CACHE[key] = _build_nc(nch_a, nch_b, nblk, use_bf16)
    nc = _NC_CACHE[key]

    fdt = ml_dtypes.bfloat16 if use_bf16 else np.float32
    NCH = nch_a + nch_b
    bones = np.kron(np.eye(32), np.ones((4, 1))).astype(fdt)
    if (NCH - 1) % 4 == 0:
        bonesl = np.zeros((128, 128), dtype=fdt)
        bonesl[:, :32] = bones
    else:
        bonesl = bones.copy()
    ident = np.eye(WIN, dtype=fdt)

    common = {
        "xext": xext,
        "bones": bones,
        "bonesl": bonesl,
        "ident": ident,
    }
    for l in range(3):
        common[f"wl{l}"] = np.asarray(inputs[f"Wl{l}"]).astype(fdt)
        wse = np.concatenate(
            [
                np.asarray(inputs[f"Ws{l}"], np.float32),
                (np.asarray(inputs[f"bl{l}"], np.float32)
                 + np.asarray(inputs[f"bs{l}"], np.float32)).reshape(1, -1),
            ],
            axis=0,
        )
        common[f"ws{l}"] = wse.astype(fdt)

    in_maps = []
    for k in range(NCORES):
        m = dict(common)
        m.update(cores[k])
        in_maps.append(m)

    from concourse.bass_utils import run_bass_kernel_spmd

    res = run_bass_kernel_spmd(nc, in_maps, core_ids=list(range(NCORES)))
    global LAST_RESULTS
    LAST_RESULTS = res
    outs = [np.asarray(res.results[k]["out"]).reshape(-1)[:SLAB]
            for k in range(NCORES)]
    return np.concatenate(outs).reshape(N, 1).astype(np.float32)


if __name__ == "__main__":
    pass



# revision 2
# speedup vs baseline: 1.4348x; 1.4348x over previous
"""Trainium2 Bass kernel for 3-layer GraphSAGE (nn_MCHCGraphSage).

Strategy (8 NeuronCores, SPMD single program):
  - Destination-sharded edges: core k owns dst nodes [k*6250, (k+1)*6250).
  - Features live in HBM as 256B rows addressed by "padded slab address"
    addr(n) = n + 22*(n//6250)  (8 slabs of 6272 rows = 50176 rows), which
    makes the inter-layer AllGather output land directly in gather space.
  - Random x[src] rows are fetched with gpsimd dma_gather (int16 indices).
    int16 range forces a two-section split: section A gathers rows
    [0, 32768), section B gathers rows [17408, 50176) (base offset 17408).
  - Segmented mean via two PE matmul levels over dst-sorted, degree-padded
    (multiple of 4) edge slots:
      level 1: constant block-ones lhsT [128, 32] sums groups of 4 slots
               into PSUM rows (two chunks per 64-group block).
      level 2: host-built 0/1 one-hot [64 groups, 128 dst] (bf16) maps
               groups to dst columns of a 128-dst window, accumulated in
               PSUM; per-group scale (1/deg) applied between levels.
  - Dense part per window, node-major: y = meanT.T @ Wl + hselfT.T @ Ws_ext
    (bias folded as an extra ones-row of hselfT), ReLU on ACT, DMA the
    [128, 64] node-major block straight to the own slab; PE-transpose to
    keep the feature-major self slab for the next layer. AllGather between
    layers redistributes slabs.
"""

import os
import sys

import numpy as np

for _p in ("/opt/trn_rl_repo", "/root/.axon_site/_ro/trn_rl_repo"):
    if os.path.isdir(_p) and _p not in sys.path:
        sys.path.append(_p)

import ml_dtypes  # noqa: E402

N = 50000
D = 64
NCORES = 8
SLAB = 6250
PSLAB = 6272
WIN = 128
NW = PSLAB // WIN  # 49
TOTROW = NCORES * PSLAB  # 50176
BBASE = TOTROW - 32768  # 17408, B-section base row
APAD_ROW = SLAB  # row 6250 (core 0 slab padding) is always zero
BPAD_ROW = 3 * PSLAB + SLAB  # row 25066 (core 3 slab padding) always zero
BW = 4  # windows per gather batch

_NC_CACHE = {}
LAST_RESULTS = None  # test harness introspection (exec_time_ns, profile)


def _addr(n):
    return n + 22 * (n // SLAB)


def _pack(x, edge_index, scale, use_bf16):
    """Host-side packing. Returns per-core dicts + structure constants."""
    src = np.asarray(edge_index[0], dtype=np.int64)
    dst = np.asarray(edge_index[1], dtype=np.int64)
    addr_e = _addr(src)

    # pass 1: global section sizes
    nch_a = 0
    nch_b = 0
    per_core = []
    for k in range(NCORES):
        sel = (dst >= k * SLAB) & (dst < (k + 1) * SLAB)
        s_k = src[sel]
        d_k = dst[sel] - k * SLAB
        a_k = addr_e[sel]
        isA = a_k <= 32767
        degA = np.bincount(d_k[isA], minlength=PSLAB)
        degB = np.bincount(d_k[~isA], minlength=PSLAB)
        padA = ((degA + 3) // 4) * 4
        padB = ((degB + 3) // 4) * 4
        wA = padA.reshape(NW, WIN).sum(1).max()
        wB = padB.reshape(NW, WIN).sum(1).max()
        nch_a = max(nch_a, (int(wA) + 127) // 128)
        nch_b = max(nch_b, (int(wB) + 127) // 128)
        per_core.append((d_k, a_k, isA, padA, padB))

    S_A = nch_a * 128
    S_B = nch_b * 128
    NCH = nch_a + nch_b
    GW = 32 * NCH  # groups per window
    NBLK = (NCH + 3) // 4  # 128-group blocks per window
    fdt = ml_dtypes.bfloat16 if use_bf16 else np.float32
    ROW = 128 if use_bf16 else 64

    # xext: node features in padded-slab address space, same for all cores
    xext = np.zeros((TOTROW, ROW), dtype=fdt)
    rows = _addr(np.arange(N))
    xext[rows, :D] = x.astype(fdt)

    cores = []
    for k in range(NCORES):
        d_k, a_k, isA, padA, padB = per_core[k]
        pA2 = padA.reshape(NW, WIN)
        pB2 = padB.reshape(NW, WIN)
        offA = (np.cumsum(pA2, 1) - pA2).reshape(-1)  # per local dst
        offB = (np.cumsum(pB2, 1) - pB2).reshape(-1)

        def build_stream(mask, off, S, base, padval):
            e_d = d_k[mask]
            e_a = a_k[mask]
            order = np.argsort(e_d, kind="stable")
            d_s = e_d[order]
            a_s = e_a[order]
            deg = np.bincount(e_d, minlength=PSLAB)
            start = np.concatenate([[0], np.cumsum(deg)])[:-1]
            rank = np.arange(len(d_s)) - start[d_s]
            pos = (d_s // WIN) * S + off[d_s] + rank
            stream = np.full(NW * S, padval, dtype=np.int64)
            stream[pos] = a_s - base
            return stream

        streamA = build_stream(isA, offA, S_A, 0, APAD_ROW)
        streamB = build_stream(~isA, offB, S_B, BBASE, BPAD_ROW - BBASE)
        assert streamA.max() <= 32767 and streamB.max() <= 32767
        assert streamA.min() >= 0 and streamB.min() >= 0

        # group -> local dst map (per window: A groups then B groups)
        grp_dst = np.full(NW * GW, -1, dtype=np.int64)
        for pad, off, sec0 in ((padA, offA, 0), (padB, offB, S_A)):
            reps = pad // 4
            tot = int(reps.sum())
            if tot == 0:
                continue
            gstart = (np.arange(PSLAB) // WIN) * GW + (sec0 + off) // 4
            base_rep = np.repeat(gstart, reps)
            cum = np.cumsum(reps) - reps
            within = np.arange(tot) - np.repeat(cum, reps)
            gpos = base_rep + within
            grp_dst[gpos] = np.repeat(np.arange(PSLAB), reps)

        valid = grp_dst >= 0
        gw = np.arange(NW * GW) // GW
        gl = np.arange(NW * GW) % GW
        onehot = np.zeros((128, NW * NBLK * 128), dtype=ml_dtypes.bfloat16)
        ocols = (gw * NBLK + gl // 128) * 128 + (grp_dst % WIN)
        onehot[gl[valid] % 128, ocols[valid]] = 1.0
        sgrp = np.zeros((128, NW * NBLK), dtype=np.float32)
        sgrp[gl[valid] % 128, (gw * NBLK + gl // 128)[valid]] = scale[
            k * SLAB + grp_dst[valid]
        ]

        stream = np.concatenate([streamA, streamB]).astype(np.int16)
        idx16 = stream.reshape(-1, 16).T.copy()  # [16, T/16]
        idx = np.tile(idx16, (8, 1))  # replicate for 8 gpsimd cores

        xselfT = np.zeros((D + 1, PSLAB), dtype=fdt)
        xselfT[:D, :SLAB] = x[k * SLAB : (k + 1) * SLAB].T.astype(fdt)
        xselfT[D, :] = 1.0  # bias row

        cores.append(
            {"idx": idx, "onehot": onehot, "sgrp": sgrp, "xselfT": xselfT}
        )

    return nch_a, nch_b, NBLK, xext, cores


def _build_nc(nch_a, nch_b, nblk, use_bf16):
    import concourse.bacc as bacc
    import concourse.tile as tile
    import concourse.mybir as mybir

    dt = mybir.dt
    fdt = dt.bfloat16 if use_bf16 else dt.float32
    ROW = 128 if use_bf16 else 64
    NCH = nch_a + nch_b
    S_A = nch_a * 128
    S_B = nch_b * 128
    T_A = NW * S_A
    T_B = NW * S_B

    nqueues = int(os.environ.get("SAGE_QUEUES", "1"))
    nc = bacc.Bacc(None, num_devices=NCORES, num_swdge_queues=nqueues)

    xext_d = nc.dram_tensor("xext", [TOTROW, ROW], fdt, kind="ExternalInput")
    idx_d = nc.dram_tensor(
        "idx", [128, (T_A + T_B) // 16], dt.int16, kind="ExternalInput"
    )
    oh_d = nc.dram_tensor(
        "onehot", [128, NW * nblk * 128], dt.bfloat16, kind="ExternalInput"
    )
    sg_d = nc.dram_tensor("sgrp", [128, NW * nblk], dt.float32, kind="ExternalInput")
    xsT_d = nc.dram_tensor("xselfT", [D + 1, PSLAB], fdt, kind="ExternalInput")
    bones_d = nc.dram_tensor("bones", [128, 32], fdt, kind="ExternalInput")
    r0_last = (NCH - 1) % 4
    M_LAST = 128 - 32 * r0_last if r0_last == 0 else 32
    bonesl_d = nc.dram_tensor("bonesl", [128, M_LAST], fdt, kind="ExternalInput")
    ident_d = nc.dram_tensor("ident", [WIN, WIN], fdt, kind="ExternalInput")
    w_d = {}
    for l, m in ((0, D), (1, D), (2, 1)):
        w_d[f"wl{l}"] = nc.dram_tensor(f"wl{l}", [D, m], fdt, kind="ExternalInput")
        w_d[f"ws{l}"] = nc.dram_tensor(
            f"ws{l}", [D + 1, m], fdt, kind="ExternalInput"
        )
    out_d = nc.dram_tensor("out", [PSLAB, 1], dt.float32, kind="ExternalOutput")

    hext_d = nc.dram_tensor("hext", [TOTROW, ROW], fdt, addr_space="Shared")
    slab_d = nc.dram_tensor("slab", [PSLAB, ROW], fdt)

    batches = []
    w0 = 0
    while w0 < NW:
        bw = min(BW, NW - w0)
        batches.append((w0, bw))
        w0 += bw
    n_layers = int(os.environ.get("SAGE_LAYERS", "3"))
    n_batch_lim = int(os.environ.get("SAGE_BATCHES", str(len(batches))))
    batches = batches[:n_batch_lim]
    no_cc = os.environ.get("SAGE_NOCC", "") == "1"

    with tile.TileContext(nc) as tc:
        with (
            tc.tile_pool(name="const", bufs=1) as cpool,
            tc.tile_pool(name="gpool", bufs=2) as gpool,
            tc.tile_pool(name="spool", bufs=3) as spool,
            tc.tile_pool(name="psA", bufs=2, space="PSUM") as psA,
            tc.tile_pool(name="psB", bufs=2, space="PSUM") as psB,
            tc.tile_pool(name="psC", bufs=2, space="PSUM") as psC,
        ):
            idx_sb = cpool.tile([128, (T_A + T_B) // 16], dt.int16, tag="idx")
            oh_sb = cpool.tile([128, NW * nblk * 128], dt.bfloat16, tag="oh")
            sg_sb = cpool.tile([128, NW * nblk], dt.float32, tag="sg")
            bones_sb = cpool.tile([128, 32], fdt, tag="bones")
            bonesl_sb = cpool.tile([128, M_LAST], fdt, tag="bonesl")
            zeros_sb = cpool.tile([128, 64], fdt, tag="zeros")
            ident_sb = cpool.tile([WIN, WIN], fdt, tag="ident")
            hs = [cpool.tile([D + 1, PSLAB], fdt, tag=f"hs{i}", name=f"hs{i}")
                  for i in range(3)]
            w_sb = {}
            for l, m in ((0, D), (1, D), (2, 1)):
                w_sb[f"wl{l}"] = cpool.tile([D, m], fdt, tag=f"wl{l}",
                                            name=f"wl{l}")
                w_sb[f"ws{l}"] = cpool.tile([D + 1, m], fdt, tag=f"ws{l}",
                                            name=f"ws{l}")
            zpad_sb = cpool.tile([PSLAB - SLAB, ROW], fdt, tag="zpad")

            nc.sync.dma_start(idx_sb[:], idx_d[:])
            nc.sync.dma_start(oh_sb[:], oh_d[:])
            nc.sync.dma_start(sg_sb[:], sg_d[:])
            nc.sync.dma_start(bones_sb[:], bones_d[:])
            nc.sync.dma_start(bonesl_sb[:], bonesl_d[:])
            nc.sync.dma_start(ident_sb[:], ident_d[:])
            nc.sync.dma_start(hs[0][:], xsT_d[:])
            for l in range(3):
                nc.sync.dma_start(w_sb[f"wl{l}"][:], w_d[f"wl{l}"][:])
                nc.sync.dma_start(w_sb[f"ws{l}"][:], w_d[f"ws{l}"][:])
            nc.vector.memset(zpad_sb[:], 0.0)
            nc.vector.memset(zeros_sb[:], 0.0)
            nc.vector.memset(hs[1][D : D + 1, :], 1.0)
            nc.vector.memset(hs[2][D : D + 1, :], 1.0)

            import contextlib
            reps = int(os.environ.get("SAGE_REPS", "1"))
            rep_cm = (tc.For_i(0, reps, 1, name="reploop")
                      if reps > 1 else contextlib.nullcontext())
            with rep_cm:
                for layer in range(n_layers):
                    src_t = xext_d if layer == 0 else hext_d
                    hself = hs[layer]
                    wl_t = w_sb[f"wl{layer}"]
                    ws_t = w_sb[f"ws{layer}"]
                    m_out = 1 if layer == 2 else D

                    for bi, (w0, bw) in enumerate(batches):
                        gA = gpool.tile([128, bw * nch_a, ROW], fdt, tag="gA")
                        gB = gpool.tile([128, bw * nch_b, ROW], fdt, tag="gB")
                        numA = bw * S_A
                        numB = bw * S_B
                        a0 = w0 * S_A // 16
                        b0c = (T_A + w0 * S_B) // 16
                        qb = 1 if os.environ.get("SAGE_QROT", "") == "1" else 0
                        nc.gpsimd.dma_gather(
                            gA[:], xext_d[:] if layer == 0 else hext_d[:],
                            idx_sb[:, a0 : a0 + numA // 16],
                            numA, numA, ROW,
                            single_packet=False,
                        )
                        nc.gpsimd.dma_gather(
                            gB[:], src_t[BBASE:, :],
                            idx_sb[:, b0c : b0c + numB // 16],
                            numB, numB, ROW,
                            single_packet=False,
                            queue_num=qb,
                        )

                        stage = int(os.environ.get("SAGE_STAGE", "9"))
                        for wi in range(bw):
                            if stage < 1:
                                break
                            w = w0 + wi
                            gsum_ps = psA.tile([128, nblk * D], dt.float32, tag="gsum")
                            # level 1: block-ones partial sums (groups of 4 slots)
                            for cc in range(NCH):
                                if cc < nch_a:
                                    rhs = gA[:, wi * nch_a + cc, 0:D]
                                else:
                                    rhs = gB[:, wi * nch_b + (cc - nch_a), 0:D]
                                blk = cc // 4
                                row = (cc % 4) * 32
                                cslice = slice(blk * D, (blk + 1) * D)
                                if cc == NCH - 1 and r0_last == 0:
                                    # covers rows [0,128): tail rows zero-filled
                                    nc.tensor.matmul(
                                        gsum_ps[0:128, cslice],
                                        bonesl_sb[:], rhs, start=True, stop=True,
                                    )
                                else:
                                    nc.tensor.matmul(
                                        gsum_ps[row : row + 32, cslice],
                                        bones_sb[:], rhs, start=True, stop=True,
                                        tile_position=(0, row),
                                    )
                                    if cc == NCH - 1 and row < 96:
                                        # zero-fill remaining rows of last block
                                        z0 = row + 32
                                        if z0 == 32:
                                            nc.tensor.matmul(
                                                gsum_ps[32:64, cslice],
                                                zeros_sb[:, 0:32], rhs,
                                                start=True, stop=True,
                                                tile_position=(0, 32),
                                            )
                                            z0 = 64
                                        if z0 == 64:
                                            nc.tensor.matmul(
                                                gsum_ps[64:128, cslice],
                                                zeros_sb[:], rhs,
                                                start=True, stop=True,
                                                tile_position=(0, 64),
                                            )
                                        elif z0 == 96:
                                            nc.tensor.matmul(
                                                gsum_ps[96:128, cslice],
                                                zeros_sb[:, 0:32], rhs,
                                                start=True, stop=True,
                                                tile_position=(0, 96),
                                            )
                            if stage < 2:
                                continue
                            # scale by 1/deg, cast to bf16 (split over ACT/DVE)
                            gsum_sb = spool.tile([128, nblk * D], dt.bfloat16,
                                                 tag="gsum_sb")
                            for blk in range(nblk):
                                sga = sg_sb[:, w * nblk + blk : w * nblk + blk + 1]
                                if blk % 2 == 0:
                                    nc.scalar.activation(
                                        gsum_sb[:, blk * D : (blk + 1) * D],
                                        gsum_ps[:, blk * D : (blk + 1) * D],
                                        mybir.ActivationFunctionType.Copy,
                                        scale=sga,
                                    )
                                else:
                                    nc.vector.tensor_scalar_mul(
                                        gsum_sb[:, blk * D : (blk + 1) * D],
                                        gsum_ps[:, blk * D : (blk + 1) * D],
                                        sga,
                                    )
                            if stage < 3:
                                continue
                            # level 2: one-hot accumulate -> meanT [D, 128] scaled
                            win_ps = psB.tile([D, WIN], dt.float32, tag="winps")
                            for blk in range(nblk):
                                oc = (w * nblk + blk) * 128
                                nc.tensor.matmul(
                                    win_ps[:],
                                    gsum_sb[:, blk * D : (blk + 1) * D],
                                    oh_sb[:, oc : oc + 128],
                                    start=(blk == 0), stop=(blk == nblk - 1),
                                )
                            if stage < 4:
                                continue
                            mean_sb = spool.tile([D, WIN], fdt, tag="mean")
                            nc.vector.tensor_copy(mean_sb[:], win_ps[:])
                            # dense, node-major: y = meanT.T@Wl + hselfT.T@Ws_ext
                            y_ps = psC.tile([WIN, m_out], dt.float32, tag="ypsum")
                            nc.tensor.matmul(y_ps[:], mean_sb[:], wl_t[:],
                                             start=True, stop=False)
                            nc.tensor.matmul(y_ps[:],
                                             hself[:, w * WIN : (w + 1) * WIN],
                                             ws_t[:], start=False, stop=True)
                            if layer < 2:
                                hn_sb = spool.tile([WIN, D], fdt, tag="hn")
                                nc.scalar.activation(
                                    hn_sb[:], y_ps[:],
                                    mybir.ActivationFunctionType.Relu,
                                )
                                nc.sync.dma_start(
                                    slab_d[w * WIN : (w + 1) * WIN, 0:D], hn_sb[:]
                                )
                                t_ps = psB.tile([D, WIN], fdt, tag="tps",
                                                name="t_ps")
                                nc.tensor.transpose(t_ps[:], hn_sb[:], ident_sb[:])
                                nc.vector.tensor_copy(
                                    hs[layer + 1][0:D, w * WIN : (w + 1) * WIN],
                                    t_ps[:],
                                )
                            else:
                                y_sb = spool.tile([WIN, 1], dt.float32, tag="ysb")
                                nc.scalar.activation(
                                    y_sb[:], y_ps[:],
                                    mybir.ActivationFunctionType.Relu,
                                )
                                nc.sync.dma_start(
                                    out_d[w * WIN : (w + 1) * WIN, :], y_sb[:]
                                )

                    if layer < 2 and layer < n_layers - 1 and not no_cc:
                        nc.sync.dma_start(slab_d[SLAB:PSLAB, :], zpad_sb[:])
                        nc.gpsimd.collective_compute(
                            "AllGather",
                            mybir.AluOpType.bypass,
                            replica_groups=[list(range(NCORES))],
                            ins=[slab_d[:]],
                            outs=[hext_d[:]],
                        )

    nc.compile()
    return nc


def kernel(**inputs):
    x = np.asarray(inputs["x"], dtype=np.float32)
    edge_index = np.asarray(inputs["edge_index"])
    use_bf16 = os.environ.get("SAGE_F32", "") != "1"

    deg = np.bincount(np.asarray(edge_index[1], dtype=np.int64), minlength=N)
    scale = np.where(deg > 0, 1.0 / np.maximum(deg, 1), 0.0).astype(np.float32)

    nch_a, nch_b, nblk, xext, cores = _pack(x, edge_index, scale, use_bf16)

    key = (nch_a, nch_b, nblk, use_bf16)
    if key not in _NC_CACHE:
        _NC_CACHE[key] = _build_nc(nch_a, nch_b, nblk, use_bf16)
    nc = _NC_CACHE[key]

    fdt = ml_dtypes.bfloat16 if use_bf16 else np.float32
    NCH = nch_a + nch_b
    bones = np.kron(np.eye(32), np.ones((4, 1))).astype(fdt)
    if (NCH - 1) % 4 == 0:
        bonesl = np.zeros((128, 128), dtype=fdt)
        bonesl[:, :32] = bones
    else:
        bonesl = bones.copy()
    ident = np.eye(WIN, dtype=fdt)

    common = {
        "xext": xext,
        "bones": bones,
        "bonesl": bonesl,
        "ident": ident,
    }
    for l in range(3):
        common[f"wl{l}"] = np.asarray(inputs[f"Wl{l}"]).astype(fdt)
        wse = np.concatenate(
            [
                np.asarray(inputs[f"Ws{l}"], np.float32),
                (np.asarray(inputs[f"bl{l}"], np.float32)
                 + np.asarray(inputs[f"bs{l}"], np.float32)).reshape(1, -1),
            ],
            axis=0,
        )
        common[f"ws{l}"] = wse.astype(fdt)

    in_maps = []
    for k in range(NCORES):
        m = dict(common)
        m.update(cores[k])
        in_maps.append(m)

    from concourse.bass_utils import run_bass_kernel_spmd

    res = run_bass_kernel_spmd(nc, in_maps, core_ids=list(range(NCORES)))
    global LAST_RESULTS
    LAST_RESULTS = res
    outs = [np.asarray(res.results[k]["out"]).reshape(-1)[:SLAB]
            for k in range(NCORES)]
    return np.concatenate(outs).reshape(N, 1).astype(np.float32)


if __name__ == "__main__":
    pass



# revision 6
# speedup vs baseline: 1.8658x; 1.3004x over previous
"""Trainium2 Bass kernel for 3-layer GraphSAGE (nn_MCHCGraphSage).

Strategy (8 NeuronCores, SPMD single program):
  - Destination-sharded edges: core k owns dst nodes [k*6250, (k+1)*6250).
  - Features live in HBM as 256B rows addressed by "padded slab address"
    addr(n) = n + 22*(n//6250)  (8 slabs of 6272 rows = 50176 rows), which
    makes the inter-layer AllGather output land directly in gather space.
  - Random x[src] rows are fetched with gpsimd dma_gather (int16 indices).
    int16 range forces a two-section split: section A gathers rows
    [0, 32768), section B gathers rows [17408, 50176) (base offset 17408).
  - Segmented mean via two PE matmul levels over dst-sorted, degree-padded
    (multiple of 4) edge slots:
      level 1: constant block-ones lhsT [128, 32] sums groups of 4 slots
               into PSUM rows (two chunks per 64-group block).
      level 2: host-built 0/1 one-hot [64 groups, 128 dst] (bf16) maps
               groups to dst columns of a 128-dst window, accumulated in
               PSUM; per-group scale (1/deg) applied between levels.
  - Dense part per window, node-major: y = meanT.T @ Wl + hselfT.T @ Ws_ext
    (bias folded as an extra ones-row of hselfT), ReLU on ACT, DMA the
    [128, 64] node-major block straight to the own slab; PE-transpose to
    keep the feature-major self slab for the next layer. AllGather between
    layers redistributes slabs.
"""

import os
import sys

import numpy as np

for _p in ("/opt/trn_rl_repo", "/root/.axon_site/_ro/trn_rl_repo"):
    if os.path.isdir(_p) and _p not in sys.path:
        sys.path.append(_p)

import ml_dtypes  # noqa: E402

N = 50000
D = 64
NCORES = 8
SLAB = 6250
PSLAB = 6272
WIN = 128
NW = PSLAB // WIN  # 49
TOTROW = NCORES * PSLAB  # 50176
BBASE = TOTROW - 32768  # 17408, B-section base row
APAD_ROW = SLAB  # row 6250 (core 0 slab padding) is always zero
BPAD_ROW = 3 * PSLAB + SLAB  # row 25066 (core 3 slab padding) always zero
BW = 4  # windows per gather batch

_NC_CACHE = {}
LAST_RESULTS = None  # test harness introspection (exec_time_ns, profile)


def _addr(n):
    return n + 22 * (n // SLAB)


def _pack(x, edge_index, scale, use_bf16):
    """Host-side packing. Returns per-core dicts + structure constants."""
    src = np.asarray(edge_index[0], dtype=np.int64)
    dst = np.asarray(edge_index[1], dtype=np.int64)
    addr_e = _addr(src)

    # pass 1: per-core A/B assignment (overlap rebalanced) + section sizes
    nch_a = 0
    nch_b = 0
    per_core = []
    for k in range(NCORES):
        sel = (dst >= k * SLAB) & (dst < (k + 1) * SLAB)
        d_k = dst[sel] - k * SLAB
        a_k = addr_e[sel]
        # A covers addr <= 32767; B covers addr >= BBASE. Overlap
        # [BBASE, 32767] is assigned per-dst to balance window A/B totals.
        forcedA = a_k < BBASE
        forcedB = a_k > 32767
        over = ~forcedA & ~forcedB
        isA = forcedA.copy()
        cntAf = np.bincount(d_k[forcedA], minlength=PSLAB)
        cntBf = np.bincount(d_k[forcedB], minlength=PSLAB)
        cntOv = np.bincount(d_k[over], minlength=PSLAB)
        p4 = lambda v: -(-v // 4) * 4
        goA = np.zeros(PSLAB, dtype=bool)
        for w in range(NW):
            totA = 0
            totB = 0
            for dloc in range(w * WIN, (w + 1) * WIN):
                aF, bF, ov = int(cntAf[dloc]), int(cntBf[dloc]), int(cntOv[dloc])
                if ov:
                    dA = p4(aF + ov) - p4(aF)  # marginal A slots
                    dB = p4(bF + ov) - p4(bF)
                    if totA + dA <= totB + dB:
                        goA[dloc] = True
                        totA += p4(aF + ov)
                        totB += p4(bF)
                    else:
                        totA += p4(aF)
                        totB += p4(bF + ov)
                else:
                    totA += p4(aF)
                    totB += p4(bF)
        isA = forcedA | (over & goA[d_k])
        degA = np.bincount(d_k[isA], minlength=PSLAB)
        degB = np.bincount(d_k[~isA], minlength=PSLAB)
        padA = ((degA + 3) // 4) * 4
        padB = ((degB + 3) // 4) * 4
        wA = padA.reshape(NW, WIN).sum(1).max()
        wB = padB.reshape(NW, WIN).sum(1).max()
        nch_a = max(nch_a, (int(wA) + 127) // 128)
        nch_b = max(nch_b, (int(wB) + 127) // 128)
        per_core.append((d_k, a_k, isA, padA, padB))

    S_A = nch_a * 128
    S_B = nch_b * 128
    NCH = nch_a + nch_b
    GW = 32 * NCH  # groups per window
    NBLK = (NCH + 3) // 4  # 128-group blocks per window
    fdt = ml_dtypes.bfloat16 if use_bf16 else np.float32
    ROW = 128 if use_bf16 else 64

    # xext: node features in padded-slab address space, same for all cores
    xext = np.zeros((TOTROW, ROW), dtype=fdt)
    rows = _addr(np.arange(N))
    xext[rows, :D] = x.astype(fdt)

    cores = []
    for k in range(NCORES):
        d_k, a_k, isA, padA, padB = per_core[k]
        pA2 = padA.reshape(NW, WIN)
        pB2 = padB.reshape(NW, WIN)
        offA = (np.cumsum(pA2, 1) - pA2).reshape(-1)  # per local dst
        offB = (np.cumsum(pB2, 1) - pB2).reshape(-1)

        def build_stream(mask, off, S, base, padval):
            e_d = d_k[mask]
            e_a = a_k[mask]
            order = np.argsort(e_d, kind="stable")
            d_s = e_d[order]
            a_s = e_a[order]
            deg = np.bincount(e_d, minlength=PSLAB)
            start = np.concatenate([[0], np.cumsum(deg)])[:-1]
            rank = np.arange(len(d_s)) - start[d_s]
            pos = (d_s // WIN) * S + off[d_s] + rank
            stream = np.full(NW * S, padval, dtype=np.int64)
            stream[pos] = a_s - base
            return stream

        streamA = build_stream(isA, offA, S_A, 0, APAD_ROW)
        streamB = build_stream(~isA, offB, S_B, BBASE, BPAD_ROW - BBASE)
        assert streamA.max() <= 32767 and streamB.max() <= 32767
        assert streamA.min() >= 0 and streamB.min() >= 0

        # group -> local dst map (per window: A groups then B groups)
        grp_dst = np.full(NW * GW, -1, dtype=np.int64)
        for pad, off, sec0 in ((padA, offA, 0), (padB, offB, S_A)):
            reps = pad // 4
            tot = int(reps.sum())
            if tot == 0:
                continue
            gstart = (np.arange(PSLAB) // WIN) * GW + (sec0 + off) // 4
            base_rep = np.repeat(gstart, reps)
            cum = np.cumsum(reps) - reps
            within = np.arange(tot) - np.repeat(cum, reps)
            gpos = base_rep + within
            grp_dst[gpos] = np.repeat(np.arange(PSLAB), reps)

        valid = grp_dst >= 0
        gw = np.arange(NW * GW) // GW
        gl = np.arange(NW * GW) % GW
        onehot = np.zeros((128, NW * NBLK * 128), dtype=ml_dtypes.bfloat16)
        ocols = (gw * NBLK + gl // 128) * 128 + (grp_dst % WIN)
        onehot[gl[valid] % 128, ocols[valid]] = 1.0
        sgrp = np.zeros((128, NW * NBLK), dtype=np.float32)
        sgrp[gl[valid] % 128, (gw * NBLK + gl // 128)[valid]] = scale[
            k * SLAB + grp_dst[valid]
        ]

        stream = np.concatenate([streamA, streamB]).astype(np.int16)
        idx16 = stream.reshape(-1, 16).T.copy()  # [16, T/16]
        idx = np.tile(idx16, (8, 1))  # replicate for 8 gpsimd cores

        xselfT = np.zeros((D + 1, PSLAB), dtype=fdt)
        xselfT[:D, :SLAB] = x[k * SLAB : (k + 1) * SLAB].T.astype(fdt)
        xselfT[D, :] = 1.0  # bias row

        cores.append(
            {"idx": idx, "onehot": onehot, "sgrp": sgrp, "xselfT": xselfT}
        )

    return nch_a, nch_b, NBLK, xext, cores


def _build_nc(nch_a, nch_b, nblk, use_bf16):
    import concourse.bacc as bacc
    import concourse.tile as tile
    import concourse.mybir as mybir

    dt = mybir.dt
    fdt = dt.bfloat16 if use_bf16 else dt.float32
    ROW = 128 if use_bf16 else 64
    NCH = nch_a + nch_b
    S_A = nch_a * 128
    S_B = nch_b * 128
    T_A = NW * S_A
    T_B = NW * S_B

    nqueues = int(os.environ.get("SAGE_QUEUES", "4"))
    nc = bacc.Bacc(None, num_devices=NCORES, num_swdge_queues=nqueues)

    xext_d = nc.dram_tensor("xext", [TOTROW, ROW], fdt, kind="ExternalInput")
    idx_d = nc.dram_tensor(
        "idx", [128, (T_A + T_B) // 16], dt.int16, kind="ExternalInput"
    )
    oh_d = nc.dram_tensor(
        "onehot", [128, NW * nblk * 128], dt.bfloat16, kind="ExternalInput"
    )
    sg_d = nc.dram_tensor("sgrp", [128, NW * nblk], dt.float32, kind="ExternalInput")
    xsT_d = nc.dram_tensor("xselfT", [D + 1, PSLAB], fdt, kind="ExternalInput")
    bones_d = nc.dram_tensor("bones", [128, 32], fdt, kind="ExternalInput")
    r0_last = (NCH - 1) % 4
    M_LAST = 128 - 32 * r0_last if r0_last == 0 else 32
    bonesl_d = nc.dram_tensor("bonesl", [128, M_LAST], fdt, kind="ExternalInput")
    ident_d = nc.dram_tensor("ident", [WIN, WIN], fdt, kind="ExternalInput")
    w_d = {}
    for l, m in ((0, D), (1, D), (2, 1)):
        w_d[f"wl{l}"] = nc.dram_tensor(f"wl{l}", [D, m], fdt, kind="ExternalInput")
        w_d[f"ws{l}"] = nc.dram_tensor(
            f"ws{l}", [D + 1, m], fdt, kind="ExternalInput"
        )
    out_d = nc.dram_tensor("out", [PSLAB, 1], dt.float32, kind="ExternalOutput")

    hext_d = nc.dram_tensor("hext", [TOTROW, ROW], fdt, addr_space="Shared")
    slab_d = nc.dram_tensor("slab", [PSLAB, ROW], fdt)

    batches = []
    w0 = 0
    while w0 < NW:
        bw = min(BW, NW - w0)
        batches.append((w0, bw))
        w0 += bw
    n_layers = int(os.environ.get("SAGE_LAYERS", "3"))
    n_batch_lim = int(os.environ.get("SAGE_BATCHES", str(len(batches))))
    batches = batches[:n_batch_lim]
    no_cc = os.environ.get("SAGE_NOCC", "") == "1"

    with tile.TileContext(nc) as tc:
        with (
            tc.tile_pool(name="const", bufs=1) as cpool,
            tc.tile_pool(name="gpool", bufs=2) as gpool,
            tc.tile_pool(name="spool", bufs=3) as spool,
            tc.tile_pool(name="psA", bufs=2, space="PSUM") as psA,
            tc.tile_pool(name="psB", bufs=2, space="PSUM") as psB,
            tc.tile_pool(name="psC", bufs=2, space="PSUM") as psC,
        ):
            idx_sb = cpool.tile([128, (T_A + T_B) // 16], dt.int16, tag="idx")
            oh_sb = cpool.tile([128, NW * nblk * 128], dt.bfloat16, tag="oh")
            sg_sb = cpool.tile([128, NW * nblk], dt.float32, tag="sg")
            bones_sb = cpool.tile([128, 32], fdt, tag="bones")
            bonesl_sb = cpool.tile([128, M_LAST], fdt, tag="bonesl")
            zeros_sb = cpool.tile([128, 64], fdt, tag="zeros")
            ident_sb = cpool.tile([WIN, WIN], fdt, tag="ident")
            hs = [cpool.tile([D + 1, PSLAB], fdt, tag=f"hs{i}", name=f"hs{i}")
                  for i in range(3)]
            w_sb = {}
            for l, m in ((0, D), (1, D), (2, 1)):
                w_sb[f"wl{l}"] = cpool.tile([D, m], fdt, tag=f"wl{l}",
                                            name=f"wl{l}")
                w_sb[f"ws{l}"] = cpool.tile([D + 1, m], fdt, tag=f"ws{l}",
                                            name=f"ws{l}")
            zpad_sb = cpool.tile([PSLAB - SLAB, ROW], fdt, tag="zpad")

            nc.sync.dma_start(idx_sb[:], idx_d[:])
            nc.sync.dma_start(oh_sb[:], oh_d[:])
            nc.sync.dma_start(sg_sb[:], sg_d[:])
            nc.sync.dma_start(bones_sb[:], bones_d[:])
            nc.sync.dma_start(bonesl_sb[:], bonesl_d[:])
            nc.sync.dma_start(ident_sb[:], ident_d[:])
            nc.sync.dma_start(hs[0][:], xsT_d[:])
            for l in range(3):
                nc.sync.dma_start(w_sb[f"wl{l}"][:], w_d[f"wl{l}"][:])
                nc.sync.dma_start(w_sb[f"ws{l}"][:], w_d[f"ws{l}"][:])
            nc.vector.memset(zpad_sb[:], 0.0)
            nc.vector.memset(zeros_sb[:], 0.0)
            nc.vector.memset(hs[1][D : D + 1, :], 1.0)
            nc.vector.memset(hs[2][D : D + 1, :], 1.0)

            import contextlib
            reps = int(os.environ.get("SAGE_REPS", "1"))
            rep_cm = (tc.For_i(0, reps, 1, name="reploop")
                      if reps > 1 else contextlib.nullcontext())
            with rep_cm:
                for layer in range(n_layers):
                    src_t = xext_d if layer == 0 else hext_d
                    hself = hs[layer]
                    wl_t = w_sb[f"wl{layer}"]
                    ws_t = w_sb[f"ws{layer}"]
                    m_out = 1 if layer == 2 else D

                    for bi, (w0, bw) in enumerate(batches):
                        gA = gpool.tile([128, bw * nch_a, ROW], fdt, tag="gA")
                        gB = gpool.tile([128, bw * nch_b, ROW], fdt, tag="gB")
                        numA = bw * S_A
                        numB = bw * S_B
                        a0 = w0 * S_A // 16
                        b0c = (T_A + w0 * S_B) // 16
                        # 4-way split across SWDGE queues (Q7 pairs 0-3).
                        # Non-blocking queues 1-3 first, blocking queue 0 last
                        # so its desc-gen overlaps pairs 1-3.
                        srcA = xext_d[:] if layer == 0 else hext_d[:]
                        hA = ((numA // 2) // 128) * 128
                        hB = ((numB // 2) // 128) * 128
                        hAc = hA // 128
                        hBc = hB // 128
                        nc.gpsimd.dma_gather(
                            gA[:, 0:hAc, :], srcA,
                            idx_sb[:, a0 : a0 + hA // 16],
                            hA, hA, ROW,
                            single_packet=False, queue_num=1,
                        )
                        nc.gpsimd.dma_gather(
                            gA[:, hAc:, :], srcA,
                            idx_sb[:, a0 + hA // 16 : a0 + numA // 16],
                            numA - hA, numA - hA, ROW,
                            single_packet=False, queue_num=2,
                        )
                        nc.gpsimd.dma_gather(
                            gB[:, 0:hBc, :], src_t[BBASE:, :],
                            idx_sb[:, b0c : b0c + hB // 16],
                            hB, hB, ROW,
                            single_packet=False, queue_num=3,
                        )
                        nc.gpsimd.dma_gather(
                            gB[:, hBc:, :], src_t[BBASE:, :],
                            idx_sb[:, b0c + hB // 16 : b0c + numB // 16],
                            numB - hB, numB - hB, ROW,
                            single_packet=False, queue_num=0,
                        )

                        stage = int(os.environ.get("SAGE_STAGE", "9"))
                        for wi in range(bw):
                            if stage < 1:
                                break
                            w = w0 + wi
                            gsum_ps = psA.tile([128, nblk * D], dt.float32, tag="gsum")
                            # level 1: block-ones partial sums (groups of 4 slots)
                            for cc in range(NCH):
                                if cc < nch_a:
                                    rhs = gA[:, wi * nch_a + cc, 0:D]
                                else:
                                    rhs = gB[:, wi * nch_b + (cc - nch_a), 0:D]
                                blk = cc // 4
                                row = (cc % 4) * 32
                                cslice = slice(blk * D, (blk + 1) * D)
                                if cc == NCH - 1 and r0_last == 0:
                                    # covers rows [0,128): tail rows zero-filled
                                    nc.tensor.matmul(
                                        gsum_ps[0:128, cslice],
                                        bonesl_sb[:], rhs, start=True, stop=True,
                                    )
                                else:
                                    nc.tensor.matmul(
                                        gsum_ps[row : row + 32, cslice],
                                        bones_sb[:], rhs, start=True, stop=True,
                                        tile_position=(0, row),
                                    )
                                    if cc == NCH - 1 and row < 96:
                                        # zero-fill remaining rows of last block
                                        z0 = row + 32
                                        if z0 == 32:
                                            nc.tensor.matmul(
                                                gsum_ps[32:64, cslice],
                                                zeros_sb[:, 0:32], rhs,
                                                start=True, stop=True,
                                                tile_position=(0, 32),
                                            )
                                            z0 = 64
                                        if z0 == 64:
                                            nc.tensor.matmul(
                                                gsum_ps[64:128, cslice],
                                                zeros_sb[:], rhs,
                                                start=True, stop=True,
                                                tile_position=(0, 64),
                                            )
                                        elif z0 == 96:
                                            nc.tensor.matmul(
                                                gsum_ps[96:128, cslice],
                                                zeros_sb[:, 0:32], rhs,
                                                start=True, stop=True,
                                                tile_position=(0, 96),
                                            )
                            if stage < 2:
                                continue
                            # scale by 1/deg, cast to bf16 (split over ACT/DVE)
                            gsum_sb = spool.tile([128, nblk * D], dt.bfloat16,
                                                 tag="gsum_sb")
                            for blk in range(nblk):
                                sga = sg_sb[:, w * nblk + blk : w * nblk + blk + 1]
                                if blk % 2 == 0:
                                    nc.scalar.activation(
                                        gsum_sb[:, blk * D : (blk + 1) * D],
                                        gsum_ps[:, blk * D : (blk + 1) * D],
                                        mybir.ActivationFunctionType.Copy,
                                        scale=sga,
                                    )
                                else:
                                    nc.vector.tensor_scalar_mul(
                                        gsum_sb[:, blk * D : (blk + 1) * D],
                                        gsum_ps[:, blk * D : (blk + 1) * D],
                                        sga,
                                    )
                            if stage < 3:
                                continue
                            # level 2: one-hot accumulate -> meanT [D, 128] scaled
                            win_ps = psB.tile([D, WIN], dt.float32, tag="winps")
                            for blk in range(nblk):
                                oc = (w * nblk + blk) * 128
                                nc.tensor.matmul(
                                    win_ps[:],
                                    gsum_sb[:, blk * D : (blk + 1) * D],
                                    oh_sb[:, oc : oc + 128],
                                    start=(blk == 0), stop=(blk == nblk - 1),
                                )
                            if stage < 4:
                                continue
                            mean_sb = spool.tile([D, WIN], fdt, tag="mean")
                            nc.vector.tensor_copy(mean_sb[:], win_ps[:])
                            # dense, node-major: y = meanT.T@Wl + hselfT.T@Ws_ext
                            y_ps = psC.tile([WIN, m_out], dt.float32, tag="ypsum")
                            nc.tensor.matmul(y_ps[:], mean_sb[:], wl_t[:],
                                             start=True, stop=False)
                            nc.tensor.matmul(y_ps[:],
                                             hself[:, w * WIN : (w + 1) * WIN],
                                             ws_t[:], start=False, stop=True)
                            if layer < 2:
                                hn_sb = spool.tile([WIN, D], fdt, tag="hn")
                                nc.scalar.activation(
                                    hn_sb[:], y_ps[:],
                                    mybir.ActivationFunctionType.Relu,
                                )
                                nc.sync.dma_start(
                                    slab_d[w * WIN : (w + 1) * WIN, 0:D], hn_sb[:]
                                )
                                t_ps = psB.tile([D, WIN], fdt, tag="tps",
                                                name="t_ps")
                                nc.tensor.transpose(t_ps[:], hn_sb[:], ident_sb[:])
                                nc.vector.tensor_copy(
                                    hs[layer + 1][0:D, w * WIN : (w + 1) * WIN],
                                    t_ps[:],
                                )
                            else:
                                y_sb = spool.tile([WIN, 1], dt.float32, tag="ysb")
                                nc.scalar.activation(
                                    y_sb[:], y_ps[:],
                                    mybir.ActivationFunctionType.Relu,
                                )
                                nc.sync.dma_start(
                                    out_d[w * WIN : (w + 1) * WIN, :], y_sb[:]
                                )

                    if layer < 2 and layer < n_layers - 1 and not no_cc:
                        nc.sync.dma_start(slab_d[SLAB:PSLAB, :], zpad_sb[:])
                        nc.gpsimd.collective_compute(
                            "AllGather",
                            mybir.AluOpType.bypass,
                            replica_groups=[list(range(NCORES))],
                            ins=[slab_d[:]],
                            outs=[hext_d[:]],
                        )

    nc.compile()
    return nc


def kernel(**inputs):
    x = np.asarray(inputs["x"], dtype=np.float32)
    edge_index = np.asarray(inputs["edge_index"])
    use_bf16 = os.environ.get("SAGE_F32", "") != "1"

    deg = np.bincount(np.asarray(edge_index[1], dtype=np.int64), minlength=N)
    scale = np.where(deg > 0, 1.0 / np.maximum(deg, 1), 0.0).astype(np.float32)

    nch_a, nch_b, nblk, xext, cores = _pack(x, edge_index, scale, use_bf16)

    key = (nch_a, nch_b, nblk, use_bf16)
    if key not in _NC_CACHE:
        _NC_CACHE[key] = _build_nc(nch_a, nch_b, nblk, use_bf16)
    nc = _NC_CACHE[key]

    fdt = ml_dtypes.bfloat16 if use_bf16 else np.float32
    NCH = nch_a + nch_b
    bones = np.kron(np.eye(32), np.ones((4, 1))).astype(fdt)
    if (NCH - 1) % 4 == 0:
        bonesl = np.zeros((128, 128), dtype=fdt)
        bonesl[:, :32] = bones
    else:
        bonesl = bones.copy()
    ident = np.eye(WIN, dtype=fdt)

    common = {
        "xext": xext,
        "bones": bones,
        "bonesl": bonesl,
        "ident": ident,
    }
    for l in range(3):
        common[f"wl{l}"] = np.asarray(inputs[f"Wl{l}"]).astype(fdt)
        wse = np.concatenate(
            [
                np.asarray(inputs[f"Ws{l}"], np.float32),
                (np.asarray(inputs[f"bl{l}"], np.float32)
                 + np.asarray(inputs[f"bs{l}"], np.float32)).reshape(1, -1),
            ],
            axis=0,
        )
        common[f"ws{l}"] = wse.astype(fdt)

    in_maps = []
    for k in range(NCORES):
        m = dict(common)
        m.update(cores[k])
        in_maps.append(m)

    from concourse.bass_utils import run_bass_kernel_spmd

    res = run_bass_kernel_spmd(nc, in_maps, core_ids=list(range(NCORES)))
    global LAST_RESULTS
    LAST_RESULTS = res
    outs = [np.asarray(res.results[k]["out"]).reshape(-1)[:SLAB]
            for k in range(NCORES)]
    return np.concatenate(outs).reshape(N, 1).astype(np.float32)


if __name__ == "__main__":
    pass



# revision 7
# speedup vs baseline: 2.1590x; 1.1571x over previous
"""Trainium2 Bass kernel for 3-layer GraphSAGE (nn_MCHCGraphSage).

Strategy (8 NeuronCores, SPMD single program):
  - Destination-sharded edges: core k owns dst nodes [k*6250, (k+1)*6250).
  - Node features stored in HBM as 256B rows in TWO half-spaces split by
    local row: half1 = slab rows [0, 3200), half2 = [3200, 6272).  Row
    addresses then fit int16 for the gpsimd dma_gather (25600 / 24576 rows).
  - Per batch of 4 dst windows, FOUR dma_gathers (half1/half2 x 2) on SWDGE
    queues 1,2,3,0: each queue uses its own Q7 core pair, so descriptor
    generation runs 4-way parallel (queue 0 dispatch blocks, so it goes
    last).
  - Aggregation: single-level one-hot matmul.  For each 128-slot chunk,
    matmul(win_ps[64, 128dst], lhsT=chunk[128slots, 64feat],
    rhs=oh[128slots, 128dst]) accumulates the *mean* directly: the host
    bakes 1/deg into the one-hot values.  Pad slots have all-zero one-hot
    rows, so no degree padding and no zero-fill matmuls are needed.
    One-hots stream from HBM per batch (double buffered).
  - Dense part per window, node-major: y = meanT.T @ Wl + hselfT.T @ Ws_ext
    (bias folded as an extra ones-row of hselfT), ReLU on ACT, PE-transpose
    to keep the feature-major self slab for the next layer.
  - Inter-layer redistribution: TWO partial AllGathers per layer (half1
    fires as soon as windows 0-24 are done, overlapping the rest of the
    layer; half2 at layer end), into per-layer-parity hext buffers.
"""

import os
import sys

import numpy as np

for _p in ("/opt/trn_rl_repo", "/root/.axon_site/_ro/trn_rl_repo"):
    if os.path.isdir(_p) and _p not in sys.path:
        sys.path.append(_p)

import ml_dtypes  # noqa: E402

N = 50000
D = 64
NCORES = 8
SLAB = 6250
PSLAB = 6272
WIN = 128
NW = PSLAB // WIN  # 49
HALF = 3200        # local rows in half1 (windows 0-24)
H2 = PSLAB - HALF  # 3072 (windows 25-48)
NW1 = HALF // WIN  # 25
H1TOT = NCORES * HALF   # 25600
H2TOT = NCORES * H2     # 24576
AZERO = H1TOT           # appended all-zero row in xext1/hext1
BZERO = 3050            # core-0 slab pad rows (local 6250) are always zero
BW = 4  # windows per gather batch

_NC_CACHE = {}
LAST_RESULTS = None  # test harness introspection (exec_time_ns, profile)


def _pack(x, edge_index, scale):
    """Host-side packing. Returns per-core dicts + structure constants."""
    src = np.asarray(edge_index[0], dtype=np.int64)
    dst = np.asarray(edge_index[1], dtype=np.int64)
    k_src = src // SLAB
    loc = src % SLAB
    isA_e = loc < HALF
    rowA_e = k_src * HALF + loc
    rowB_e = k_src * H2 + (loc - HALF)

    # pass 1: section sizes
    nch_a = 0
    nch_b = 0
    per_core = []
    for k in range(NCORES):
        sel = (dst >= k * SLAB) & (dst < (k + 1) * SLAB)
        d_k = dst[sel] - k * SLAB
        isA = isA_e[sel]
        row_k = np.where(isA, rowA_e[sel], rowB_e[sel])
        degA = np.bincount(d_k[isA], minlength=PSLAB)
        degB = np.bincount(d_k[~isA], minlength=PSLAB)
        wA = degA.reshape(NW, WIN).sum(1).max()
        wB = degB.reshape(NW, WIN).sum(1).max()
        nch_a = max(nch_a, (int(wA) + 127) // 128)
        nch_b = max(nch_b, (int(wB) + 127) // 128)
        per_core.append((d_k, row_k, isA, degA, degB))

    S_A = nch_a * 128
    S_B = nch_b * 128
    NCH = nch_a + nch_b
    T_A = NW * S_A
    T_B = NW * S_B
    fdt = ml_dtypes.bfloat16
    ROW = 128

    # node features in the two half-spaces (256B rows, 64 used)
    nodes = np.arange(N)
    nloc = nodes % SLAB
    nk = nodes // SLAB
    xext1 = np.zeros((H1TOT + 1, ROW), dtype=fdt)
    xext2 = np.zeros((H2TOT, ROW), dtype=fdt)
    m1 = nloc < HALF
    xext1[(nk * HALF + nloc)[m1], :D] = x[m1].astype(fdt)
    xext2[(nk * H2 + nloc - HALF)[~m1], :D] = x[~m1].astype(fdt)

    cores = []
    for k in range(NCORES):
        d_k, row_k, isA, degA, degB = per_core[k]
        offA = degA.reshape(NW, WIN)
        offA = (np.cumsum(offA, 1) - offA).reshape(-1)
        offB = degB.reshape(NW, WIN)
        offB = (np.cumsum(offB, 1) - offB).reshape(-1)

        def build(mask, deg, off, S, padval):
            e_d = d_k[mask]
            e_r = row_k[mask]
            order = np.argsort(e_d, kind="stable")
            d_s = e_d[order]
            r_s = e_r[order]
            start = np.concatenate([[0], np.cumsum(deg)])[:-1]
            rank = np.arange(len(d_s)) - start[d_s]
            pos = (d_s // WIN) * S + off[d_s] + rank
            stream = np.full(NW * S, padval, dtype=np.int64)
            stream[pos] = r_s
            return stream, pos, d_s

        streamA, posA, dA = build(isA, degA, offA, S_A, AZERO)
        streamB, posB, dB = build(~isA, degB, offB, S_B, BZERO)
        assert streamA.max() <= AZERO and streamB.max() < H2TOT
        assert streamA.min() >= 0 and streamB.min() >= 0

        # one-hot with 1/deg folded in: [128 slot-in-chunk, (w*NCH+cc)*128+dst]
        oh = np.zeros((128, NW * NCH * 128), dtype=fdt)
        sc_k = scale[k * SLAB : (k + 1) * SLAB]
        scp = np.zeros(PSLAB, dtype=np.float32)
        scp[:SLAB] = sc_k
        for pos, d_s, cc0, S in ((posA, dA, 0, S_A), (posB, dB, nch_a, S_B)):
            w = pos // S
            r = pos % S
            cc = cc0 + r // 128
            prow = r % 128
            col = (w * NCH + cc) * 128 + (d_s % WIN)
            oh[prow, col] = scp[d_s]

        stream = np.concatenate([streamA, streamB]).astype(np.int16)
        idx16 = stream.reshape(-1, 16).T.copy()  # [16, T/16]
        idx = np.tile(idx16, (8, 1))  # replicate for 8 gpsimd cores

        xselfT = np.zeros((D + 1, PSLAB), dtype=fdt)
        xselfT[:D, :SLAB] = x[k * SLAB : (k + 1) * SLAB].T.astype(fdt)
        xselfT[D, :] = 1.0  # bias row

        cores.append({"idx": idx, "onehot": oh, "xselfT": xselfT})

    return nch_a, nch_b, xext1, xext2, cores


def _build_nc(nch_a, nch_b):
    import concourse.bacc as bacc
    import concourse.tile as tile
    import concourse.mybir as mybir

    dt = mybir.dt
    fdt = dt.bfloat16
    ROW = 128
    NCH = nch_a + nch_b
    S_A = nch_a * 128
    S_B = nch_b * 128
    T_A = NW * S_A
    T_B = NW * S_B

    nqueues = int(os.environ.get("SAGE_QUEUES", "4"))
    nc = bacc.Bacc(None, num_devices=NCORES, num_swdge_queues=nqueues)

    xe1_d = nc.dram_tensor("xext1", [H1TOT + 1, ROW], fdt, kind="ExternalInput")
    xe2_d = nc.dram_tensor("xext2", [H2TOT, ROW], fdt, kind="ExternalInput")
    idx_d = nc.dram_tensor(
        "idx", [128, (T_A + T_B) // 16], dt.int16, kind="ExternalInput"
    )
    oh_d = nc.dram_tensor(
        "onehot", [128, NW * NCH * 128], dt.bfloat16, kind="ExternalInput"
    )
    xsT_d = nc.dram_tensor("xselfT", [D + 1, PSLAB], fdt, kind="ExternalInput")
    ident_d = nc.dram_tensor("ident", [WIN, WIN], fdt, kind="ExternalInput")
    w_d = {}
    for l, m in ((0, D), (1, D), (2, 1)):
        w_d[f"wl{l}"] = nc.dram_tensor(f"wl{l}", [D, m], fdt, kind="ExternalInput")
        w_d[f"ws{l}"] = nc.dram_tensor(
            f"ws{l}", [D + 1, m], fdt, kind="ExternalInput"
        )
    out_d = nc.dram_tensor("out", [PSLAB, 1], dt.float32, kind="ExternalOutput")

    he1 = [nc.dram_tensor(f"hext1{p}", [H1TOT + 1, ROW], fdt, addr_space="Shared")
           for p in "ab"]
    he2 = [nc.dram_tensor(f"hext2{p}", [H2TOT, ROW], fdt, addr_space="Shared")
           for p in "ab"]
    slab1_d = nc.dram_tensor("slab1", [HALF, ROW], fdt)
    slab2_d = nc.dram_tensor("slab2", [H2, ROW], fdt)

    batches = []
    w0 = 0
    while w0 < NW:
        bw = min(BW, NW - w0)
        batches.append((w0, bw))
        w0 += bw

    with tile.TileContext(nc) as tc:
        with (
            tc.tile_pool(name="const", bufs=1) as cpool,
            tc.tile_pool(name="gpool", bufs=2) as gpool,
            tc.tile_pool(name="ohpool", bufs=2) as ohpool,
            tc.tile_pool(name="spool", bufs=3) as spool,
            tc.tile_pool(name="psA", bufs=4, space="PSUM") as psA,
            tc.tile_pool(name="psB", bufs=2, space="PSUM") as psB,
            tc.tile_pool(name="psC", bufs=2, space="PSUM") as psC,
        ):
            idx_sb = cpool.tile([128, (T_A + T_B) // 16], dt.int16, tag="idx")
            ident_sb = cpool.tile([WIN, WIN], fdt, tag="ident")
            zrow_sb = cpool.tile([1, ROW], fdt, tag="zrow")
            zpad_sb = cpool.tile([PSLAB - SLAB, ROW], fdt, tag="zpad")
            hs = [cpool.tile([D + 1, PSLAB], fdt, tag=f"hs{i}", name=f"hs{i}")
                  for i in range(3)]
            w_sb = {}
            for l, m in ((0, D), (1, D), (2, 1)):
                w_sb[f"wl{l}"] = cpool.tile([D, m], fdt, tag=f"wl{l}",
                                            name=f"wl{l}")
                w_sb[f"ws{l}"] = cpool.tile([D + 1, m], fdt, tag=f"ws{l}",
                                            name=f"ws{l}")

            nc.sync.dma_start(idx_sb[:], idx_d[:])
            nc.sync.dma_start(ident_sb[:], ident_d[:])
            nc.sync.dma_start(hs[0][:], xsT_d[:])
            for l in range(3):
                nc.sync.dma_start(w_sb[f"wl{l}"][:], w_d[f"wl{l}"][:])
                nc.sync.dma_start(w_sb[f"ws{l}"][:], w_d[f"ws{l}"][:])
            nc.vector.memset(zrow_sb[:], 0.0)
            nc.vector.memset(zpad_sb[:], 0.0)
            nc.vector.memset(hs[1][D : D + 1, :], 1.0)
            nc.vector.memset(hs[2][D : D + 1, :], 1.0)
            # appended zero rows of the hext1 buffers
            nc.sync.dma_start(he1[0][H1TOT : H1TOT + 1, :], zrow_sb[:])
            nc.sync.dma_start(he1[1][H1TOT : H1TOT + 1, :], zrow_sb[:])

            for layer in range(3):
                if layer == 0:
                    srcA_t, srcB_t = xe1_d, xe2_d
                else:
                    srcA_t, srcB_t = he1[layer - 1], he2[layer - 1]
                hself = hs[layer]
                wl_t = w_sb[f"wl{layer}"]
                ws_t = w_sb[f"ws{layer}"]
                m_out = 1 if layer == 2 else D

                for bi, (w0, bw) in enumerate(batches):
                    gA = gpool.tile([128, bw * nch_a, ROW], fdt, tag="gA")
                    gB = gpool.tile([128, bw * nch_b, ROW], fdt, tag="gB")
                    oh_sb = ohpool.tile([128, bw * NCH * 128], dt.bfloat16,
                                        tag="oh")
                    nc.sync.dma_start(
                        oh_sb[:],
                        oh_d[:, w0 * NCH * 128 : (w0 + bw) * NCH * 128],
                    )
                    numA = bw * S_A
                    numB = bw * S_B
                    a0 = w0 * S_A // 16
                    b0c = (T_A + w0 * S_B) // 16
                    hA = ((numA // 2) // 128) * 128
                    hB = ((numB // 2) // 128) * 128
                    hAc = hA // 128
                    hBc = hB // 128
                    nc.gpsimd.dma_gather(
                        gA[:, 0:hAc, :], srcA_t[:],
                        idx_sb[:, a0 : a0 + hA // 16],
                        hA, hA, ROW,
                        single_packet=False, queue_num=1,
                    )
                    nc.gpsimd.dma_gather(
                        gA[:, hAc:, :], srcA_t[:],
                        idx_sb[:, a0 + hA // 16 : a0 + numA // 16],
                        numA - hA, numA - hA, ROW,
                        single_packet=False, queue_num=2,
                    )
                    nc.gpsimd.dma_gather(
                        gB[:, 0:hBc, :], srcB_t[:],
                        idx_sb[:, b0c : b0c + hB // 16],
                        hB, hB, ROW,
                        single_packet=False, queue_num=3,
                    )
                    nc.gpsimd.dma_gather(
                        gB[:, hBc:, :], srcB_t[:],
                        idx_sb[:, b0c + hB // 16 : b0c + numB // 16],
                        numB - hB, numB - hB, ROW,
                        single_packet=False, queue_num=0,
                    )

                    for wi in range(bw):
                        w = w0 + wi
                        win_ps = psA.tile([D, WIN], dt.float32, tag="winps")
                        for cc in range(NCH):
                            if cc < nch_a:
                                lhsT = gA[:, wi * nch_a + cc, 0:D]
                            else:
                                lhsT = gB[:, wi * nch_b + (cc - nch_a), 0:D]
                            oc = (wi * NCH + cc) * 128
                            nc.tensor.matmul(
                                win_ps[:], lhsT, oh_sb[:, oc : oc + 128],
                                start=(cc == 0), stop=(cc == NCH - 1),
                            )
                        mean_sb = spool.tile([D, WIN], fdt, tag="mean")
                        nc.vector.tensor_copy(mean_sb[:], win_ps[:])
                        y_ps = psC.tile([WIN, m_out], dt.float32, tag="ypsum")
                        nc.tensor.matmul(y_ps[:], mean_sb[:], wl_t[:],
                                         start=True, stop=False)
                        nc.tensor.matmul(y_ps[:],
                                         hself[:, w * WIN : (w + 1) * WIN],
                                         ws_t[:], start=False, stop=True)
                        if layer < 2:
                            hn_sb = spool.tile([WIN, D], fdt, tag="hn")
                            nc.scalar.activation(
                                hn_sb[:], y_ps[:],
                                mybir.ActivationFunctionType.Relu,
                            )
                            if w < NW1:
                                nc.sync.dma_start(
                                    slab1_d[w * WIN : (w + 1) * WIN, 0:D],
                                    hn_sb[:],
                                )
                            else:
                                r0 = w * WIN - HALF
                                nc.sync.dma_start(
                                    slab2_d[r0 : r0 + WIN, 0:D], hn_sb[:]
                                )
                            t_ps = psB.tile([D, WIN], fdt, tag="tps",
                                            name="t_ps")
                            nc.tensor.transpose(t_ps[:], hn_sb[:], ident_sb[:])
                            nc.vector.tensor_copy(
                                hs[layer + 1][0:D, w * WIN : (w + 1) * WIN],
                                t_ps[:],
                            )
                        else:
                            y_sb = spool.tile([WIN, 1], dt.float32, tag="ysb")
                            nc.scalar.activation(
                                y_sb[:], y_ps[:],
                                mybir.ActivationFunctionType.Relu,
                            )
                            nc.sync.dma_start(
                                out_d[w * WIN : (w + 1) * WIN, :], y_sb[:]
                            )

                    # half1 AllGather fires as soon as windows 0-24 are done
                    if layer < 2 and w0 + bw - 1 >= NW1 - 1 and w0 < NW1:
                        nc.gpsimd.collective_compute(
                            "AllGather",
                            mybir.AluOpType.bypass,
                            replica_groups=[list(range(NCORES))],
                            ins=[slab1_d[:]],
                            outs=[he1[layer][0:H1TOT, :]],
                        )

                if layer < 2:
                    nc.sync.dma_start(slab2_d[SLAB - HALF :, :], zpad_sb[:])
                    nc.gpsimd.collective_compute(
                        "AllGather",
                        mybir.AluOpType.bypass,
                        replica_groups=[list(range(NCORES))],
                        ins=[slab2_d[:]],
                        outs=[he2[layer][:]],
                    )

    nc.compile()
    return nc


def kernel(**inputs):
    x = np.asarray(inputs["x"], dtype=np.float32)
    edge_index = np.asarray(inputs["edge_index"])

    deg = np.bincount(np.asarray(edge_index[1], dtype=np.int64), minlength=N)
    scale = np.where(deg > 0, 1.0 / np.maximum(deg, 1), 0.0).astype(np.float32)

    nch_a, nch_b, xext1, xext2, cores = _pack(x, edge_index, scale)

    key = (nch_a, nch_b)
    if key not in _NC_CACHE:
        _NC_CACHE[key] = _build_nc(nch_a, nch_b)
    nc = _NC_CACHE[key]

    fdt = ml_dtypes.bfloat16
    ident = np.eye(WIN, dtype=fdt)

    common = {"xext1": xext1, "xext2": xext2, "ident": ident}
    for l in range(3):
        common[f"wl{l}"] = np.asarray(inputs[f"Wl{l}"]).astype(fdt)
        wse = np.concatenate(
            [
                np.asarray(inputs[f"Ws{l}"], np.float32),
                (np.asarray(inputs[f"bl{l}"], np.float32)
                 + np.asarray(inputs[f"bs{l}"], np.float32)).reshape(1, -1),
            ],
            axis=0,
        )
        common[f"ws{l}"] = wse.astype(fdt)

    in_maps = []
    for k in range(NCORES):
        m = dict(common)
        m.update(cores[k])
        m["idx"] = cores[k]["idx"]
        m["onehot"] = cores[k]["onehot"]
        in_maps.append(m)

    from concourse.bass_utils import run_bass_kernel_spmd

    res = run_bass_kernel_spmd(nc, in_maps, core_ids=list(range(NCORES)))
    global LAST_RESULTS
    LAST_RESULTS = res
    outs = [np.asarray(res.results[k]["out"]).reshape(-1)[:SLAB]
            for k in range(NCORES)]
    return np.concatenate(outs).reshape(N, 1).astype(np.float32)


if __name__ == "__main__":
    pass
